# revision 31
# baseline (speedup 1.0000x reference)
"""Causal multi-head attention (B=4, L=2048, D=1024, H=16, HD=64) on 8 TRN2
NeuronCores.

Sharding: core c handles batch b = c//2 and head-group g = c%2 (8 heads =
512 output dims). Attention is fully independent per (b, h); no collectives.

Per-core device kernel:
  - bf16 matmul operands on the projection/S path (full-rate PE streaming at
    every moving-dim size, Fast Weight Load, half the DMA bytes); fp32r on
    the es/PV path (ScalarE writes f32r ~20% faster than bf16, and fp32r
    matmuls stream full-rate at N>=256).
  - X^T resident in SBUF; weights stream as [128, 1536] (Wq|Wk|Wv) tiles.
    Input DMAs are chopped per 512-l chunk / per weight band and spread over
    the sync/scalar/gpsimd queues so the first Q matmul starts ~4us after
    data flow begins.
  - Q^T, K^T with head_dim on partitions; K^T zero-padded to K=128 (pad
    written once by GpSimd memset) so every attention matmul keeps one PE
    row configuration (row-config mode switches drain the array, ~107ns).
  - V in natural [l, dim] layout with a ones column per head: the PV matmul
    accumulates softmax denominators into row 64 of O^T_aug. bv is added
    during the PSUM drain against a host-broadcast [128, 512] bias tile.
  - S^T[m, q] = K^T.T @ Q^T per (head, q-chunk 512, m-tile 128); blocks
    above the causal diagonal are skipped; exp(0.25*s) on ScalarE with the
    scale fused; diagonal blocks masked by a 0/1 multiply after exp.
    No max-subtraction: logits are O(10) so fp32 exp cannot overflow.
  - O^T_aug[65, q] accumulates over m-tiles in PSUM, drains to bf16 and DMAs
    out in transposed layout; the host divides rows 0:64 by the row-64
    denominator and transposes back while unsharding (host-side layout +
    final normalization scaling; all matmuls, exp and reductions on device).
  - QKV projection (phase B) and attention (phase C) interleave in emission
    order: C(qc) is paced against B(qc+1) slices (B(3)'s V slices pace into
    C(3), which only needs them from m-tile 12 on) so ScalarE's exp stream
    overlaps projection matmuls instead of serializing after them.
"""

import sys

if "/opt/trn_rl_repo" not in sys.path:
    sys.path.insert(0, "/opt/trn_rl_repo")

import numpy as np
import ml_dtypes

import concourse.bass as bass  # noqa: F401
import concourse.bacc as bacc
import concourse.tile as tile
from concourse import mybir
from concourse.bass_utils import run_bass_kernel_spmd

B, L, D = 4, 2048, 1024
H, HD = 16, 64
NCORES = 8
DIMS = 512  # output dims per core (8 heads)
NKT = 8  # k-tiles over D
NDT = 4  # dim-tiles over DIMS
NQC = 4  # q-chunks of 512
NLT = 16  # l-tiles of 128
SCALE = 0.25  # 1/sqrt(H)
BF16 = mybir.dt.bfloat16
F32R = mybir.dt.float32r
F32 = mybir.dt.float32
AF = mybir.ActivationFunctionType

_cache = {}


def _build_kernel(es_bufs=4, sps_bufs=2, interleave=True, pv_stagger=2, split_exp=False):
    nc = bacc.Bacc("TRN2", target_bir_lowering=False, debug=False)

    XT = nc.declare_dram_parameter("XT", [D, L], BF16, isOutput=False)
    # WALL = [WqT | WkT | WvT] concatenated on the output-dim axis.
    WALL = nc.declare_dram_parameter("WALL", [D, 3 * DIMS], BF16, isOutput=False)
    # packed constants: CONSTR = [ones8 | mask], CONSTF = [bq | bk | bvb]
    CONSTR = nc.declare_dram_parameter("CONSTR", [128, 136], F32R, isOutput=False)
    CONSTF = nc.declare_dram_parameter("CONSTF", [128, 520], F32, isOutput=False)
    # O^T_aug per head: rows 0:64 numerators, row 64 denominators.
    OUTT = nc.declare_dram_parameter("OUTT", [8, 65, L], BF16, isOutput=True)

    with tile.TileContext(nc) as tc:
        with tc.tile_pool(name="persist", bufs=1) as pp:
            # ---- input DMAs: packed constants first, then bulk, 3 queues ----
            constr = pp.tile([128, 136], F32R, tag="constr", name="constr")
            nc.scalar.dma_start(out=constr, in_=CONSTR[:, :])
            constf = pp.tile([128, 520], F32, tag="constf", name="constf")
            nc.scalar.dma_start(out=constf, in_=CONSTF[:, :])
            # preload the exp table set during the DMA-wait window
            scr = pp.tile([128, 1], F32R, tag="scr", name="scr")
            nc.scalar.activation(scr, constr[:, 0:1], AF.Exp)

            xt = [pp.tile([128, L], BF16, tag=f"xt{k}", name=f"xt{k}") for k in range(NKT)]
            for lc in range(NQC):  # chunk-major so B(0) is fed first
                lsl = slice(lc * 512, (lc + 1) * 512)
                for k in range(NKT):
                    nc.sync.dma_start(
                        out=xt[k][:, lsl], in_=XT[k * 128 : (k + 1) * 128, lsl]
                    )
            wall = [
                pp.tile([128, 3 * DIMS], BF16, tag=f"w{k}", name=f"w{k}")
                for k in range(NKT)
            ]
            for k in range(NKT):  # q band on scalar (behind the two consts)
                nc.scalar.dma_start(
                    out=wall[k][:, 0:512], in_=WALL[k * 128 : (k + 1) * 128, 0:512]
                )
            for k in range(NKT):  # k band on gpsimd
                nc.gpsimd.dma_start(
                    out=wall[k][:, 512:1024], in_=WALL[k * 128 : (k + 1) * 128, 512:1024]
                )
            for k in range(NKT):  # v band split across both queues
                eng = nc.gpsimd if k % 2 == 0 else nc.scalar
                eng.dma_start(
                    out=wall[k][:, 1024:1536], in_=WALL[k * 128 : (k + 1) * 128, 1024:1536]
                )

            # ---- persistent intermediates ----
            qt = [pp.tile([128, L], BF16, tag=f"qt{d}", name=f"qt{d}") for d in range(NDT)]
            ktp = [pp.tile([128, L], BF16, tag=f"ktp{h}", name=f"ktp{h}") for h in range(8)]
            vaug = [pp.tile([128, 8, 65], F32R, tag=f"va{t}", name=f"va{t}") for t in range(NLT)]
            for t in range(NLT):
                nc.vector.tensor_copy(
                    vaug[t][:, :, 64:65],
                    constr[:, 0:8].rearrange("p (h o) -> p h o", o=1),
                )

            with (
                tc.tile_pool(name="psB", bufs=2, space="PSUM") as psB,
                tc.tile_pool(name="psS", bufs=sps_bufs, space="PSUM") as psS,
                tc.tile_pool(name="psO", bufs=1, space="PSUM") as psO,
                tc.tile_pool(name="esb", bufs=es_bufs) as esb,
                tc.tile_pool(name="fin", bufs=2) as fin,
            ):
                # ---------- phase B emitters ----------
                def emit_q_slice(lc, d):
                    lsl = slice(lc * 512, (lc + 1) * 512)
                    dsl = slice(d * 128, (d + 1) * 128)
                    q_ps = psB.tile([128, 512], F32, tag="pb", bufs=2, name="psq")
                    for k in range(NKT):
                        nc.tensor.matmul(
                            q_ps, wall[k][:, dsl], xt[k][:, lsl],
                            start=(k == 0), stop=(k == NKT - 1),
                        )
                    nc.vector.tensor_scalar_add(qt[d][:, lsl], q_ps, constf[:, d : d + 1])

                def emit_k_slice(lc, d):
                    lsl = slice(lc * 512, (lc + 1) * 512)
                    k_ps = psB.tile([128, 512], F32, tag="pb", bufs=2, name="psk")
                    for k in range(NKT):
                        nc.tensor.matmul(
                            k_ps, wall[k][:, 512 + d * 128 : 512 + (d + 1) * 128],
                            xt[k][:, lsl],
                            start=(k == 0), stop=(k == NKT - 1),
                        )
                    nc.vector.tensor_scalar_add(
                        ktp[2 * d][0:64, lsl], k_ps[0:64, :], constf[0:64, 4 + d : 5 + d]
                    )
                    nc.vector.tensor_scalar_add(
                        ktp[2 * d + 1][64:128, lsl], k_ps[64:128, :],
                        constf[64:128, 4 + d : 5 + d],
                    )
                    if lc == 0:  # zero this pair's K pad halves once
                        nc.vector.memset(ktp[2 * d][64:128, :], 0.0)
                        nc.vector.memset(ktp[2 * d + 1][0:64, :], 0.0)

                def emit_v_slice(lc, lb):
                    lt = lc * 4 + lb
                    v_ps = psB.tile([128, 512], F32, tag="pb", bufs=2, name="psv")
                    for k in range(NKT):
                        nc.tensor.matmul(
                            v_ps, xt[k][:, lt * 128 : (lt + 1) * 128],
                            wall[k][:, 1024:1536],
                            start=(k == 0), stop=(k == NKT - 1),
                        )
                    nc.vector.tensor_add(
                        vaug[lt][:, :, 0:64],
                        v_ps[:].rearrange("p (h d) -> p h d", h=8),
                        constf[:, 8:520].rearrange("p (h d) -> p h d", h=8),
                    )

                # ---------- phase C emitters ----------
                def emit_c_block(qc, hp, mt):
                    qsl0 = qc * 512
                    msl = slice(mt * 128, (mt + 1) * 128)
                    off = mt * 128 - qc * 512
                    o = max(0, off)
                    vsa = slice(o, 512)
                    vsb = slice(512 + o, 1024)
                    qv = slice(qsl0 + o, qsl0 + 512)
                    s_ps = psS.tile([128, 1024], F32, tag="sps", name="sps")
                    nc.tensor.matmul(
                        s_ps[:, vsa], ktp[2 * hp][:, msl], qt[hp][:, qv],
                        start=True, stop=True,
                    )
                    nc.tensor.matmul(
                        s_ps[:, vsb], ktp[2 * hp + 1][:, msl], qt[hp][:, qv],
                        start=True, stop=True,
                    )
                    es = esb.tile([128, 1024], F32R, tag="es", name="es")
                    if split_exp or o > 128:
                        nc.scalar.activation(es[:, vsa], s_ps[:, vsa], AF.Exp, scale=SCALE)
                        nc.scalar.activation(es[:, vsb], s_ps[:, vsb], AF.Exp, scale=SCALE)
                    else:
                        nc.scalar.activation(
                            es[:, o:1024], s_ps[:, o:1024], AF.Exp, scale=SCALE
                        )
                    if off >= 0:  # triangular 128-col edge of the block
                        w = min(o + 128, 512) - o
                        nc.vector.tensor_mul(
                            es[:, o : o + w], es[:, o : o + w], constr[:, 8 : 8 + w]
                        )
                        nc.vector.tensor_mul(
                            es[:, 512 + o : 512 + o + w],
                            es[:, 512 + o : 512 + o + w],
                            constr[:, 8 : 8 + w],
                        )
                    return es, o

                def emit_c_pv(hp, mt, nmt, po_a, po_b, es, o):
                    vsa = slice(o, 512)
                    vsb = slice(512 + o, 1024)
                    nc.tensor.matmul(
                        po_a[:, vsa], vaug[mt][:, 2 * hp, :], es[:, vsa],
                        start=(mt == 0), stop=(mt == nmt - 1),
                    )
                    nc.tensor.matmul(
                        po_b[:, slice(o, 512)], vaug[mt][:, 2 * hp + 1, :], es[:, vsb],
                        start=(mt == 0), stop=(mt == nmt - 1),
                    )

                def emit_c_finalize(qc, hp, po_a, po_b):
                    for half, po in ((0, po_a), (1, po_b)):
                        h = 2 * hp + half
                        ot = fin.tile([65, 512], BF16, tag="ot", bufs=4, name=f"ot{h}")
                        nc.vector.tensor_copy(ot, po)
                        nc.sync.dma_start(
                            out=OUTT[h, :, qc * 512 : (qc + 1) * 512], in_=ot
                        )

                # HAM warm-up: fp32 matmuls on the constant tile while the
                # PE would otherwise idle waiting for X/W DMAs.  ~7us of
                # array activity flips the clock gate to 8/8 before the
                # first real projection matmul issues.
                warm = psS.tile([128, 1024], F32, tag="sps", name="warm")
                for i in range(8):
                    nc.tensor.matmul(
                        warm[:, 0:512], constf[:, 0:128], constf[:, 0:512],
                        start=True, stop=True,
                    )

                # ---------- interleaved emission ----------
                # B(qc) slices emit inside C(qc): q/k one hp AHEAD of the
                # attention blocks that read them (so S never waits on a
                # freshly-emitted DVE drain), V just before hp=0's diagonal
                # blocks (the first PV consumers).  PE never idles waiting
                # for a whole projection phase, and the exp stream starts
                # ~15us in.
                seq = [(qc, hp) for qc in range(NQC) for hp in range(4)]
                if interleave:
                    emit_q_slice(0, 0)
                    emit_k_slice(0, 0)
                for si, (qc, hp) in enumerate(seq):
                    nmt = 4 * qc + 4
                    if not interleave and hp == 0:
                        for d in range(NDT):
                            emit_q_slice(qc, d)
                            emit_k_slice(qc, d)
                        for lb in range(4):
                            emit_v_slice(qc, lb)
                    po_a = psO.tile([65, 512], F32, tag="poa", name="poa")
                    po_b = psO.tile([65, 512], F32, tag="pob", name="pob")
                    pend = []
                    for mt in range(nmt):
                        if interleave and hp == 0 and mt >= 4 * qc:
                            emit_v_slice(qc, mt - 4 * qc)
                        es, o = emit_c_block(qc, hp, mt)
                        pend.append((mt, es, o))
                        if interleave and mt == 1 and si + 1 < len(seq):
                            nqc, nhp = seq[si + 1]
                            emit_q_slice(nqc, nhp)
                            emit_k_slice(nqc, nhp)
                        if len(pend) > pv_stagger:
                            m0, e0, o0 = pend.pop(0)
                            emit_c_pv(hp, m0, nmt, po_a, po_b, e0, o0)
                    for m0, e0, o0 in pend:
                        emit_c_pv(hp, m0, nmt, po_a, po_b, e0, o0)
                    emit_c_finalize(qc, hp, po_a, po_b)

    nc.compile()
    return nc


def _host_inputs(X, Wq, bq, Wk, bk, Wv, bv):
    """Build the 8 per-core input maps (host-side sharding + layout prep)."""
    X = np.asarray(X, dtype=np.float32)
    Wq = np.asarray(Wq, dtype=np.float32)
    Wk = np.asarray(Wk, dtype=np.float32)
    Wv = np.asarray(Wv, dtype=np.float32)
    bq = np.asarray(bq, dtype=np.float32)
    bk = np.asarray(bk, dtype=np.float32)
    bv = np.asarray(bv, dtype=np.float32)

    bf = ml_dtypes.bfloat16
    mask = (np.arange(128)[None, :] >= np.arange(128)[:, None]).astype(np.float32)
    constr = np.concatenate([np.ones((128, 8), dtype=np.float32), mask], axis=1)

    in_maps = []
    for c in range(NCORES):
        b, g = divmod(c, 2)
        dsl = slice(g * DIMS, (g + 1) * DIMS)
        wall = np.concatenate(
            [Wq[dsl, :].T, Wk[dsl, :].T, Wv[dsl, :].T], axis=1
        ).astype(bf)
        constf = np.concatenate(
            [
                bq[dsl].reshape(128, NDT, order="F"),  # [128, 4] col d = bq d-tile
                bk[dsl].reshape(128, NDT, order="F"),
                np.tile(bv[dsl][None, :], (128, 1)),
            ],
            axis=1,
        ).astype(np.float32)
        in_maps.append(
            {
                "XT": np.ascontiguousarray(X[b].T).astype(bf),
                "WALL": np.ascontiguousarray(wall),
                "CONSTR": np.ascontiguousarray(constr),
                "CONSTF": np.ascontiguousarray(constf),
            }
        )
    return in_maps


def _assemble(res):
    """Host epilogue: normalize by the denominator row and transpose back."""
    out = np.empty((B, L, D), dtype=np.float32)
    for c in range(NCORES):
        b, g = divmod(c, 2)
        o = np.asarray(res.results[c]["OUTT"], dtype=np.float32)  # [8, 65, L]
        r = o[:, 0:64, :] / o[:, 64:65, :]  # [8, 64, L]
        out[b, :, g * DIMS : (g + 1) * DIMS] = (
            r.transpose(2, 0, 1).reshape(L, DIMS)
        )
    return out


def _run(in_maps, trace=False, variant=None):
    key = ("nc", variant)
    if key not in _cache:
        kw = dict(VARIANTS.get(variant, {}))
        _cache[key] = _build_kernel(**kw)
    res = run_bass_kernel_spmd(
        _cache[key], in_maps, core_ids=list(range(NCORES)), trace=trace
    )
    return res


VARIANTS = {
    None: {},
    "noil": {"interleave": False},
    "esb6": {"es_bufs": 6},
    "stag1": {"pv_stagger": 1},
    "stag3": {"pv_stagger": 3},
    "splitexp": {"split_exp": True},
}


def kernel(X, Wq, bq, Wk, bk, Wv, bv):
    in_maps = _host_inputs(X, Wq, bq, Wk, bk, Wv, bv)
    res = _run(in_maps, trace=False)
    return _assemble(res)
